# revision 1
# baseline (speedup 1.0000x reference)
"""NVFP4Linear (fused RMSNorm + NVFP4 quant-dequant + GEMM) on 8 TRN2 cores.

Self-contained Bass/Tile kernel.  kernel(x, weight) takes the FULL inputs
(x [16384,4096] bf16, weight [4096,4096] bf16) and returns the FULL
[16384,4096] fp32 output.

Sharding: x rows data-parallel (2048/core).  Weight quantization is sharded
(512 rows/core) and the dequantized bf16 weight is AllGathered on-device;
each core then computes its 2048x4096x4096 GEMM on the TensorEngine in bf16
(dequantized NVFP4 values are exactly representable in bf16: fp4 e2m1 has 1
mantissa bit, the e4m3 scale has 3, so products carry <=5 significant bits).

Quantization math runs on the VectorEngine in fp32 with exact bit tricks:
  - e4m3 roundtrip of amax/6: grid rounding  (v + h) - h  where
    h = 1.5*2^23 * u and u = max(2^floor(log2 v) / 8, 2^-9)  (RNE, matches
    OCP e4m3fn incl. subnormals exactly; verified vs ml_dtypes on 400k pts).
  - e2m1 quantize of ax = v/scale: same trick with u = max(2^e(ax),1)/2.
    The weight path additionally multiplies ax by (1+2^-23) and clamps to
    +-6 to reproduce the reference's ties-up rounding and ax<=6 clamp
    (bf16 weights x e4m3 scales hit exact ties ~1% of the time).
"""

import sys

for _p in ("/opt/trn_rl_repo", "/root/.axon_site/_ro/trn_rl_repo"):
    if _p not in sys.path:
        sys.path.append(_p)

import numpy as np
import concourse.bass as bass  # noqa: F401  (registers engines)
import concourse.mybir as mybir
import concourse.tile as tile
from concourse import bacc
from concourse.bass_utils import run_bass_kernel_spmd

dt = mybir.dt
Alu = mybir.AluOpType
Act = mybir.ActivationFunctionType

BLK = 16
EXP_MASK = 0x7F800000
F4_EXP_MIN = 0x3F800000
F4_H_ADD = 0x0B400000
E4M3_EXP_SUB = 0x01800000
E4M3_U_MIN = 0x3B000000
E4M3_H_ADD = 0x0BC00000

N_CORES = 8
M_FULL, K, N = 16384, 4096, 4096
M_SHARD = M_FULL // N_CORES
W_SHARD = N // N_CORES
M_GROUP = 4
N_CHUNK = 256


def _build():
    KH = K // 2
    KT = K // 128
    M_tiles = M_SHARD // 128
    W_tiles = W_SHARD // 128
    N_chunks = N // N_CHUNK
    G = M_tiles // M_GROUP

    nc = bacc.Bacc("TRN2", target_bir_lowering=False, debug=False,
                   num_devices=N_CORES)

    x_sh = nc.declare_dram_parameter("x_sh", [M_SHARD, K], dt.bfloat16,
                                     isOutput=False)
    w_sh = nc.declare_dram_parameter("w_sh", [W_SHARD, K], dt.bfloat16,
                                     isOutput=False)
    out = nc.declare_dram_parameter("out", [M_SHARD, N], dt.float32,
                                    isOutput=True)

    # weight gathered PRE-TRANSPOSED: core c contributes qw^T[:, 512c:512c+512]
    # as a [K, W_SHARD] block; AllGather stacks blocks along axis 0.
    wq_locT = nc.dram_tensor("wq_locT", [K, W_SHARD], dt.bfloat16)
    wq_fullT = nc.dram_tensor("wq_fullT", [N_CORES * K, W_SHARD], dt.bfloat16,
                              addr_space="Shared")

    with tile.TileContext(nc) as tc:
        with (
            tc.tile_pool(name="src", bufs=3) as p_src,
            tc.tile_pool(name="f32a", bufs=2) as p_f32a,
            tc.tile_pool(name="f32b", bufs=2) as p_f32b,
            tc.tile_pool(name="qout", bufs=3) as p_q,
            tc.tile_pool(name="small", bufs=3) as p_sm,
            tc.tile_pool(name="row", bufs=3) as p_row,
            tc.tile_pool(name="qxT", bufs=2 * M_GROUP) as p_qxT,
            tc.tile_pool(name="qwT", bufs=2) as p_qwT,
            tc.tile_pool(name="stage", bufs=6) as p_stage,
            tc.tile_pool(name="psum", bufs=6, space="PSUM") as p_psum,
        ):
            def quant_half(src_half, q_half, inv_rms_ap):
                nbh = src_half.shape[-1] // BLK
                src_blk = src_half.rearrange("p (b s) -> p b s", s=BLK)

                amax = p_sm.tile([128, nbh], dt.float32, tag="amax")
                nc.vector.tensor_reduce(
                    out=amax[:], in_=src_blk, axis=mybir.AxisListType.X,
                    op=Alu.max, apply_absolute_value=True)

                v = p_sm.tile([128, nbh], dt.float32, tag="v")
                if inv_rms_ap is not None:
                    nc.vector.tensor_scalar(
                        out=v[:], in0=amax[:], scalar1=inv_rms_ap,
                        scalar2=float(np.float32(1.0 / 6.0)),
                        op0=Alu.mult, op1=Alu.mult)
                else:
                    nc.vector.tensor_scalar(
                        out=v[:], in0=amax[:],
                        scalar1=float(np.float32(1.0 / 6.0)), scalar2=None,
                        op0=Alu.mult)

                h8 = p_sm.tile([128, nbh], dt.float32, tag="h8")
                nc.vector.tensor_scalar(
                    out=h8[:].bitcast(dt.int32), in0=v[:].bitcast(dt.int32),
                    scalar1=EXP_MASK, scalar2=None, op0=Alu.bitwise_and)
                nc.vector.tensor_scalar(
                    out=h8[:].bitcast(dt.int32), in0=h8[:].bitcast(dt.int32),
                    scalar1=E4M3_H_ADD - E4M3_EXP_SUB,
                    scalar2=E4M3_U_MIN + E4M3_H_ADD,
                    op0=Alu.add, op1=Alu.max)
                scal = p_sm.tile([128, nbh], dt.float32, tag="scal")
                nc.vector.tensor_tensor(out=scal[:], in0=v[:], in1=h8[:],
                                        op=Alu.add)
                nc.vector.tensor_tensor(out=scal[:], in0=scal[:], in1=h8[:],
                                        op=Alu.subtract)

                g = p_sm.tile([128, nbh], dt.float32, tag="g")
                nc.vector.reciprocal(g[:], scal[:])
                if inv_rms_ap is not None:
                    nc.vector.tensor_scalar(
                        out=g[:], in0=g[:], scalar1=inv_rms_ap,
                        scalar2=1.0e30, op0=Alu.mult, op1=Alu.min)
                else:
                    nc.vector.tensor_scalar(
                        out=g[:], in0=g[:], scalar1=1.0e30, scalar2=None,
                        op0=Alu.min)

                ax = p_f32a.tile([128, KH], dt.float32, tag="ax")
                axv = ax[:, :src_half.shape[-1]]
                axb_sb = axv.rearrange("p (b s) -> p s b", s=BLK)
                src_sb = src_half.rearrange("p (b s) -> p s b", s=BLK)
                nc.vector.tensor_tensor(
                    out=axb_sb, in0=src_sb,
                    in1=g[:, None, :].broadcast_to([128, BLK, nbh]),
                    op=Alu.mult)

                if inv_rms_ap is None:
                    nc.vector.tensor_scalar(
                        out=axv, in0=axv,
                        scalar1=float(np.float32(1.0 + 2.0 ** -23)),
                        scalar2=6.0, op0=Alu.mult, op1=Alu.min)
                    nc.vector.tensor_scalar(
                        out=axv, in0=axv, scalar1=-6.0, scalar2=None,
                        op0=Alu.max)

                h = p_f32b.tile([128, KH], dt.float32, tag="h")
                hv = h[:, :src_half.shape[-1]]
                nc.vector.tensor_scalar(
                    out=hv.bitcast(dt.int32), in0=axv.bitcast(dt.int32),
                    scalar1=EXP_MASK, scalar2=None, op0=Alu.bitwise_and)
                nc.vector.tensor_scalar(
                    out=hv.bitcast(dt.int32), in0=hv.bitcast(dt.int32),
                    scalar1=F4_H_ADD, scalar2=F4_EXP_MIN + F4_H_ADD,
                    op0=Alu.add, op1=Alu.max)
                nc.vector.tensor_tensor(out=axv, in0=axv, in1=hv, op=Alu.add)
                nc.vector.tensor_tensor(out=axv, in0=axv, in1=hv,
                                        op=Alu.subtract)
                qb_sb = q_half.rearrange("p (b s) -> p s b", s=BLK)
                nc.vector.tensor_tensor(
                    out=qb_sb, in0=axb_sb,
                    in1=scal[:, None, :].broadcast_to([128, BLK, nbh]),
                    op=Alu.mult)

            def quant_tile(dram_src, row0, with_rms):
                src = p_src.tile([128, K], dt.bfloat16, tag="src")
                nc.sync.dma_start(out=src[:], in_=dram_src[row0:row0 + 128, :])

                inv_rms_ap = None
                if with_rms:
                    # Sum(x^2) on the (otherwise idle) ScalarEngine:
                    # Square activation with fused row-accumulate.
                    ssum = p_row.tile([128, 2], dt.float32, tag="ssum")
                    for hi, h0 in enumerate((0, K // 2)):
                        sq = p_f32a.tile([128, K // 2], dt.float32, tag="ax")
                        nc.scalar.activation(
                            out=sq[:], in_=src[:, h0:h0 + K // 2],
                            func=Act.Square,
                            accum_out=ssum[:, hi:hi + 1])
                    ssum2 = p_row.tile([128, 1], dt.float32, tag="ssum2")
                    nc.vector.tensor_reduce(
                        out=ssum2[:], in_=ssum[:],
                        axis=mybir.AxisListType.X, op=Alu.add)
                    ms = p_row.tile([128, 1], dt.float32, tag="ms")
                    nc.vector.tensor_scalar(
                        out=ms[:], in0=ssum2[:],
                        scalar1=float(np.float32(1.0 / K)), scalar2=1e-6,
                        op0=Alu.mult, op1=Alu.add)
                    srms = p_row.tile([128, 1], dt.float32, tag="srms")
                    nc.scalar.activation(out=srms[:], in_=ms[:], func=Act.Sqrt)
                    invr = p_row.tile([128, 1], dt.float32, tag="invr")
                    nc.vector.reciprocal(invr[:], srms[:])
                    inv_rms_ap = invr[:]

                q = p_q.tile([128, K], dt.bfloat16, tag="q")
                quant_half(src[:, :K // 2], q[:, :K // 2], inv_rms_ap)
                quant_half(src[:, K // 2:], q[:, K // 2:], inv_rms_ap)
                return q

            wlT = wq_locT.ap().rearrange("(kc kp) n -> kp kc n", kp=128)
            for wt in range(W_tiles):
                q = quant_tile(w_sh, wt * 128, with_rms=False)
                t = p_qxT.tile([128, KT, 128], dt.bfloat16, tag="qxT")
                nc.sync.dma_start_transpose(out=t[:], in_=q[:])
                nc.sync.dma_start(out=wlT[:, :, wt * 128:(wt + 1) * 128],
                                  in_=t[:])

            nc.gpsimd.collective_compute(
                "AllGather", Alu.bypass,
                replica_groups=[list(range(N_CORES))],
                ins=[wq_locT.ap().opt()], outs=[wq_fullT.ap().opt()])

            qxT = {}

            def quant_group(g):
                for mt in range(g * M_GROUP, (g + 1) * M_GROUP):
                    q = quant_tile(x_sh, mt * 128, with_rms=True)
                    t = p_qxT.tile([128, KT, 128], dt.bfloat16, tag="qxT")
                    nc.sync.dma_start_transpose(out=t[:], in_=q[:])
                    qxT[mt] = t

            wfT = wq_fullT.ap().rearrange("(c kc kp) n -> c kp kc n",
                                          kp=128, kc=KT)

            def gemm_group(g):
                for n in range(N_chunks):
                    wchunk = p_qwT.tile([128, KT, N_CHUNK], dt.bfloat16,
                                        tag="qwT")
                    n0 = n * N_CHUNK
                    c, col0 = n0 // W_SHARD, n0 % W_SHARD
                    nc.sync.dma_start(
                        out=wchunk[:],
                        in_=wfT[c][:, :, col0:col0 + N_CHUNK])
                    for mt in range(g * M_GROUP, (g + 1) * M_GROUP):
                        ps = p_psum.tile([128, N_CHUNK], dt.float32, tag="ps")
                        for k in range(KT):
                            nc.tensor.matmul(
                                ps[:], lhsT=qxT[mt][:, k, :],
                                rhs=wchunk[:, k, :],
                                start=(k == 0), stop=(k == KT - 1))
                        st = p_stage.tile([128, N_CHUNK], dt.float32,
                                          tag="st")
                        nc.scalar.copy(st[:], ps[:])
                        nc.sync.dma_start(
                            out=out[mt * 128:(mt + 1) * 128,
                                    n * N_CHUNK:(n + 1) * N_CHUNK],
                            in_=st[:])

            quant_group(0)
            for g in range(G):
                if g + 1 < G:
                    quant_group(g + 1)
                gemm_group(g)

    nc.compile()
    return nc


_NC = None


def kernel(x, weight):
    global _NC
    if _NC is None:
        _NC = _build()

    x = np.ascontiguousarray(x)
    weight = np.ascontiguousarray(weight)
    in_maps = [
        {"x_sh": x[c * M_SHARD:(c + 1) * M_SHARD],
         "w_sh": weight[c * W_SHARD:(c + 1) * W_SHARD]}
        for c in range(N_CORES)
    ]
    res = run_bass_kernel_spmd(_NC, in_maps, list(range(N_CORES)))
    return np.concatenate([res.results[c]["out"] for c in range(N_CORES)],
                          axis=0)

